# revision 1
# baseline (speedup 1.0000x reference)
"""Trainium2 Bass kernel for nn_DenseCoordination (gnn_message_passing).

Math (per batch b):
    hi = s @ W1a ; hj = s @ W1b                       [N, 2D]
    q[i,j,:] = (s_i * s_j) @ W1c + hi_i + hj_j + b1   [N, N, 2D]
    logits[i,j] = relu(q[i,j,:]) @ W2 + b2
    w = softmax(mask(logits), axis=-1) (nan_to_num)
    ctx = w @ s ; gate = ones

Sharding: 8 cores = 4 batches x 2 i-halves. Each core owns b = c//2 and
i in [128*(c%2), 128*(c%2)+128), computes its [128, N] logits / w / ctx.

v2 changes vs baseline:
  - Hot-loop matmuls in bf16 (both operands) so the stationary operand gets
    a separate, overlapped LDWEIGHTS instead of the fp32r self-loading path.
  - Columns permuted pos-first (|W2| folded in, as before). Per j:
      * PE: 2 H-matmuls (prod @ W1c', N=512) + one-hot ident matmul adding
        hj'_j + b1' to all 512 cols + identity matmul adding hi' to the
        neg cols only.
      * DVE: pos cols via the max-trick (relu(H+hi) = max(H,-hi)+hi),
        one scalar_tensor_tensor with accum -> accp[:, j].
      * ACT: neg cols via scalar.activation(Relu, accum_out) on the full
        preactivation -> acca[:, j].
    logits = accp + sum_pos hi' - acca.
  - Prods computed in bf16 (bf16 in/out tensor_scalar) for the faster DVE
    mode; host supplies bf16 copies of s-derived tensors and W1c'.
"""

import sys

sys.path.insert(0, "/opt/trn_rl_repo")

import numpy as np
import ml_dtypes

import concourse.bacc as bacc
import concourse.bass as bass
import concourse.tile as tile
from concourse import mybir
from concourse.bass_utils import run_bass_kernel_spmd

D = 256
N = 256
B = 4
H2 = 512  # 2*D
NI = 128  # i rows per core
N_CORES = 8
F32 = mybir.dt.float32
F32R = mybir.dt.float32r
BF16 = mybir.dt.bfloat16
NEG_BIG = -1.0e30

_BUILD_CACHE: dict = {}


def _build(npos: int, with_loop: bool = True):
    AF = mybir.ActivationFunctionType
    ALU = mybir.AluOpType
    cP = npos          # pos cols -> DVE (max-trick)
    cA = H2 - npos     # neg cols -> ACT (relu + accum)

    nc = bacc.Bacc("TRN2", target_bir_lowering=False, debug=False,
                   num_devices=N_CORES)

    s_in = nc.dram_tensor("s", [N, D], F32, kind="ExternalInput").ap()
    sT_in = nc.dram_tensor("sT", [D, N], F32, kind="ExternalInput").ap()
    sTi_in = nc.dram_tensor("sTi", [D, NI], F32, kind="ExternalInput").ap()
    sTib_in = nc.dram_tensor("sTib", [D, NI], BF16, kind="ExternalInput").ap()
    w1a_in = nc.dram_tensor("W1a", [D, H2], F32, kind="ExternalInput").ap()
    w1b_in = nc.dram_tensor("W1b", [D, H2], F32, kind="ExternalInput").ap()
    w1cb_in = nc.dram_tensor("W1cb", [D, H2], BF16, kind="ExternalInput").ap()
    b1_in = nc.dram_tensor("b1", [1, H2], F32, kind="ExternalInput").ap()
    madd_in = nc.dram_tensor("madd", [NI, N], F32, kind="ExternalInput").ap()
    rowind_in = nc.dram_tensor("rowind", [NI, 1], F32, kind="ExternalInput").ap()
    ident_in = nc.dram_tensor("ident", [128, 128], F32, kind="ExternalInput").ap()
    identb_in = nc.dram_tensor("identb", [128, 128], BF16,
                               kind="ExternalInput").ap()
    nrep_in = nc.dram_tensor("nrep", [1, 1], mybir.dt.int32,
                             kind="ExternalInput").ap()
    w_out = nc.dram_tensor("w", [NI, N], F32, kind="ExternalOutput").ap()
    ctx_out = nc.dram_tensor("ctx", [NI, D], F32, kind="ExternalOutput").ap()

    with tile.TileContext(nc) as tc:
        with (
            tc.tile_pool(name="persist", bufs=1) as pp,
            tc.tile_pool(name="prod", bufs=12) as prodp,
            tc.tile_pool(name="trash", bufs=4) as trp,
            tc.tile_pool(name="psum", bufs=6, space="PSUM") as psp,
            tc.tile_pool(name="psum2", bufs=1, space="PSUM") as psp2,
            tc.tile_pool(name="psum3", bufs=1, space="PSUM") as psp3,
            tc.tile_pool(name="small", bufs=2) as smp,
        ):
            def body(_iv=None):
                # ---- load inputs into SBUF
                s_sb = []
                sT_sb = []
                sTi_sb = []
                sTib_sb = []
                w1a_sb = []
                w1b_sb = []
                w1cb_sb = []
                for c in range(2):
                    t = pp.tile([128, N], F32, tag=f"s{c}")
                    nc.sync.dma_start(t[:], s_in[128 * c:128 * c + 128, :])
                    s_sb.append(t)
                    t = pp.tile([128, N], F32, tag=f"sT{c}")
                    nc.sync.dma_start(t[:], sT_in[128 * c:128 * c + 128, :])
                    sT_sb.append(t)
                    t = pp.tile([128, NI], F32, tag=f"sTi{c}")
                    nc.sync.dma_start(t[:], sTi_in[128 * c:128 * c + 128, :])
                    sTi_sb.append(t)
                    t = pp.tile([128, NI], BF16, tag=f"sTib{c}")
                    nc.sync.dma_start(t[:], sTib_in[128 * c:128 * c + 128, :])
                    sTib_sb.append(t)
                    for nm, src, lst, dt_ in (("a", w1a_in, w1a_sb, F32),
                                              ("b", w1b_in, w1b_sb, F32),
                                              ("cb", w1cb_in, w1cb_sb, BF16)):
                        t = pp.tile([128, H2], dt_, tag=f"W1{nm}{c}")
                        nc.sync.dma_start(t[:], src[128 * c:128 * c + 128, :])
                        lst.append(t)
                b1_sb = pp.tile([1, H2], F32, tag="b1")
                nc.sync.dma_start(b1_sb[:], b1_in[:])
                madd_sb = pp.tile([NI, N], F32, tag="madd")
                nc.sync.dma_start(madd_sb[:], madd_in[:])
                rowind_sb = pp.tile([NI, 1], F32, tag="rowind")
                nc.sync.dma_start(rowind_sb[:], rowind_in[:])
                ident_sb = pp.tile([128, 128], F32, tag="ident")
                nc.sync.dma_start(ident_sb[:], ident_in[:])
                identb_sb = pp.tile([128, 128], BF16, tag="identb")
                nc.sync.dma_start(identb_sb[:], identb_in[:])
                ones_sb = pp.tile([128, 128], F32, tag="ones")
                nc.gpsimd.memset(ones_sb[:], 1.0)
                sT_r = []
                sTi_r = []
                w1a_r = []
                w1b_r = []
                for c in range(2):
                    t = pp.tile([128, N], F32R, tag=f"sTr{c}")
                    nc.vector.tensor_copy(t[:], sT_sb[c][:])
                    sT_r.append(t)
                    t = pp.tile([128, NI], F32R, tag=f"sTir{c}")
                    nc.vector.tensor_copy(t[:], sTi_sb[c][:])
                    sTi_r.append(t)
                    t = pp.tile([128, H2], F32R, tag=f"W1ar{c}")
                    nc.vector.tensor_copy(t[:], w1a_sb[c][:])
                    w1a_r.append(t)
                    t = pp.tile([128, H2], F32R, tag=f"W1br{c}")
                    nc.vector.tensor_copy(t[:], w1b_sb[c][:])
                    w1b_r.append(t)
                b1_r = pp.tile([1, H2], F32R, tag="b1r")
                nc.vector.tensor_copy(b1_r[:], b1_sb[:])
                ones_r = pp.tile([128, 128], F32R, tag="onesr")
                nc.vector.tensor_copy(ones_r[:], ones_sb[:])

                # ---- setup: HJ' = s @ W1b' + b1'  (2 chunks of 128 j's)
                hj_bf = []
                for jc in range(2):
                    ps = psp.tile([128, H2], F32, tag="ps")
                    for kc in range(2):
                        nc.tensor.matmul(
                            ps[:], sT_r[kc][:, 128 * jc:128 * jc + 128],
                            w1b_r[kc][:], start=(kc == 0), stop=False)
                    nc.tensor.matmul(ps[:], ones_r[0:1, :], b1_r[0:1, :],
                                     start=False, stop=True)
                    t = pp.tile([128, H2], BF16, tag=f"hjb{jc}")
                    nc.scalar.copy(t[:], ps[:])
                    hj_bf.append(t)

                # ---- setup: HI' = s[i-range] @ W1a' (no b1)
                ps = psp.tile([128, H2], F32, tag="ps")
                for kc in range(2):
                    nc.tensor.matmul(ps[:], sTi_r[kc][:], w1a_r[kc][:],
                                     start=(kc == 0), stop=(kc == 1))
                hi_sb = pp.tile([128, H2], F32, tag="hi")
                nc.scalar.copy(hi_sb[:], ps[:])
                hi_bf = pp.tile([128, H2], BF16, tag="hib")
                nc.vector.tensor_copy(hi_bf[:], hi_sb[:])
                neghi_sb = pp.tile([128, H2], F32, tag="neghi")
                nc.vector.tensor_scalar_mul(neghi_sb[:], hi_sb[:], -1.0)
                # (sum_pos hi' is constant per row i -> cancels in softmax)

                # ---- main fixed-j loop
                accp = pp.tile([NI, N], F32, tag="accp")
                acca = pp.tile([NI, N], F32, tag="acca")
                for j in range(N):
                    jc, jr = j // 128, j % 128
                    pt = []
                    for kc in range(2):
                        t = prodp.tile([128, NI], BF16, tag=f"pt{kc}")
                        nc.vector.tensor_scalar_mul(
                            t[:], sTib_sb[kc][:],
                            sT_sb[kc][:, j:j + 1])
                        pt.append(t)
                    ps = psp.tile([128, H2], F32, tag="ps")
                    for kc in range(2):
                        nc.tensor.matmul(
                            ps[:], pt[kc][:], w1cb_sb[kc][:],
                            start=(kc == 0), stop=False)
                    # += hj'_j + b1' on all cols (one-hot row jr broadcast)
                    nc.tensor.matmul(
                        ps[:],
                        identb_sb[:, jr:jr + 1].to_broadcast((128, 128)),
                        hj_bf[jc][:],
                        start=False, stop=(cA == 0))
                    if cA > 0:
                        # += hi' on the ACT (neg) cols
                        nc.tensor.matmul(
                            ps[:, cP:H2], identb_sb[:], hi_bf[:, cP:H2],
                            start=False, stop=True)
                    if cA > 0:
                        tr2 = psp3.tile([128, cA], F32, tag="tr2")
                        nc.scalar.activation(tr2[:], ps[:, cP:H2], AF.Relu,
                                             accum_out=acca[:, j:j + 1])
                    else:
                        nc.gpsimd.memset(acca[:, j:j + 1], 0.0)
                    if cP > 0:
                        tr1 = trp.tile([128, cP], F32, tag="tr1")
                        nc.vector.scalar_tensor_tensor(
                            out=tr1[:], in0=ps[:, 0:cP], scalar=0.0,
                            in1=neghi_sb[:, 0:cP], op0=ALU.add,
                            op1=ALU.max, accum_out=accp[:, j:j + 1])
                    else:
                        nc.gpsimd.memset(accp[:, j:j + 1], 0.0)

                # logits = accp - acca (per-row constants cancel in softmax)
                logits = pp.tile([NI, N], F32, tag="logits")
                nc.vector.tensor_sub(logits[:], accp[:], acca[:])

                # ---- masked softmax over j
                l2 = pp.tile([NI, N], F32, tag="l2")
                nc.vector.tensor_add(l2[:], logits[:], madd_sb[:])
                negm = smp.tile([NI, 1], F32, tag="negm")
                nc.vector.tensor_reduce(negm[:], l2[:],
                                        axis=mybir.AxisListType.X, op=ALU.max,
                                        negate=True)
                ex = pp.tile([NI, N], F32, tag="ex")
                ssum = smp.tile([NI, 1], F32, tag="ssum")
                nc.scalar.activation(ex[:], l2[:], AF.Exp, bias=negm[:, 0:1],
                                     accum_out=ssum[:, 0:1])
                rec = smp.tile([NI, 1], F32, tag="rec")
                nc.vector.reciprocal(rec[:], ssum[:])
                rec2 = smp.tile([NI, 1], F32, tag="rec2")
                nc.vector.tensor_mul(rec2[:], rec[:], rowind_sb[:])
                w_sb = pp.tile([NI, N], F32, tag="wsb")
                nc.vector.tensor_scalar_mul(w_sb[:], ex[:], rec2[:, 0:1])

                # ---- ctx = w @ s  (transpose w on the PE first)
                wt_sb = []
                for jc in range(2):
                    pst = psp2.tile([128, D], F32, tag="tail")
                    nc.tensor.transpose(pst[:, 0:128],
                                        w_sb[:, 128 * jc:128 * jc + 128],
                                        ident_sb[:])
                    t = smp.tile([128, 128], F32, tag=f"wt{jc}")
                    nc.vector.tensor_copy(t[:], pst[:, 0:128])
                    wt_sb.append(t)
                psc = psp2.tile([128, D], F32, tag="tail")
                for jc in range(2):
                    nc.tensor.matmul(psc[:], wt_sb[jc][:], s_sb[jc][:],
                                     start=(jc == 0), stop=(jc == 1))
                ctx_sb = pp.tile([NI, D], F32, tag="ctxsb")
                nc.scalar.copy(ctx_sb[:], psc[:])

                # ---- outputs
                nc.sync.dma_start(w_out[:], w_sb[:])
                nc.sync.dma_start(ctx_out[:], ctx_sb[:])

            if with_loop:
                nrep_sb = pp.tile([1, 1], mybir.dt.int32, tag="nrep")
                nc.sync.dma_start(nrep_sb[:], nrep_in[:])
                rv = nc.values_load(nrep_sb[0:1, 0:1], min_val=1,
                                    max_val=100000,
                                    skip_runtime_bounds_check=True)
                with tc.For_i(0, rv, 1):
                    body()
            else:
                body()

    nc.compile()
    return nc


def _prep(s, W1, b1, W2, b2, adj_allowed, active_mask, act_mask):
    s = np.ascontiguousarray(np.asarray(s, dtype=np.float32))
    W1 = np.asarray(W1, dtype=np.float32)
    b1 = np.asarray(b1, dtype=np.float32).reshape(-1)
    W2 = np.asarray(W2, dtype=np.float32).reshape(-1)  # [2D]
    adj = np.asarray(adj_allowed)
    am = np.asarray(active_mask)
    km = np.asarray(act_mask)

    pos = W2 >= 0.0
    perm = np.concatenate([np.nonzero(pos)[0], np.nonzero(~pos)[0]])
    npos = int(pos.sum())
    w2p = np.abs(W2[perm])
    W1a = np.ascontiguousarray(W1[:D][:, perm] * w2p[None, :])
    W1b = np.ascontiguousarray(W1[D:2 * D][:, perm] * w2p[None, :])
    W1c = np.ascontiguousarray(W1[2 * D:][:, perm] * w2p[None, :])
    b1p = np.ascontiguousarray((b1[perm] * w2p)[None, :])

    valid = (adj > 0) & (am > 0)[:, None, :] & (km > 0)[:, :, None]
    madd = np.where(valid, np.float32(0.0), np.float32(NEG_BIG))
    rowind = valid.any(axis=-1).astype(np.float32)
    ident = np.eye(128, dtype=np.float32)
    return s, W1a, W1b, W1c, b1p, madd, rowind, ident, npos


def _in_maps(s, W1a, W1b, W1c, b1p, madd, rowind, ident, nrep):
    nrep_arr = np.full((1, 1), nrep, dtype=np.int32)
    w1cb = W1c.astype(ml_dtypes.bfloat16)
    identb = ident.astype(ml_dtypes.bfloat16)
    maps = []
    for c in range(N_CORES):
        b, i0 = c // 2, NI * (c % 2)
        sb = s[b]
        sTb = np.ascontiguousarray(sb.T)
        sTb_bf = sTb.astype(ml_dtypes.bfloat16)
        maps.append({
            "s": sb,
            "sT": sTb,
            "sTi": np.ascontiguousarray(sTb[:, i0:i0 + NI]),
            "sTib": np.ascontiguousarray(sTb_bf[:, i0:i0 + NI]),
            "W1a": W1a, "W1b": W1b, "W1cb": w1cb, "b1": b1p,
            "madd": np.ascontiguousarray(madd[b, i0:i0 + NI]),
            "rowind": np.ascontiguousarray(rowind[b, i0:i0 + NI, None]),
            "ident": ident, "identb": identb,
            "nrep": nrep_arr,
        })
    return maps


def _gather(results):
    w = np.empty((B, N, N), dtype=np.float32)
    ctx = np.empty((B, N, D), dtype=np.float32)
    for c in range(N_CORES):
        b, i0 = c // 2, NI * (c % 2)
        w[b, i0:i0 + NI] = results[c]["w"]
        ctx[b, i0:i0 + NI] = results[c]["ctx"]
    gate = np.ones((B, N, N), dtype=np.float32)
    return ctx, gate, w


def _get_program(npos, with_loop=True):
    key = (npos, with_loop)
    if key not in _BUILD_CACHE:
        _BUILD_CACHE[key] = _build(npos, with_loop=with_loop)
    return _BUILD_CACHE[key]


def run(nrep, *, with_loop=True, **inputs):
    """Run the device kernel with the compute body repeated `nrep` times."""
    s, W1a, W1b, W1c, b1p, madd, rowind, ident, npos = _prep(**inputs)
    nc = _get_program(npos, with_loop=with_loop)
    maps = _in_maps(s, W1a, W1b, W1c, b1p, madd, rowind, ident, nrep)
    res = run_bass_kernel_spmd(nc, maps, list(range(N_CORES)))
    return _gather(res.results)


def kernel(**inputs):
    return run(1, **inputs)

